# revision 31
# baseline (speedup 1.0000x reference)
"""CMMD loss kernel for Trainium2 (Bass/Tile), 8-core SPMD.

Math (reference semantics):
  X = concat(source, target)            [N, D]
  L2[i,j] = ||X_i - X_j||^2  (via Gram trick)
  bw  = sum(L2) / (N^2 - N) / 4
  K   = sum_{l=0..4} exp(-L2 / (bw * 2^l))
  loss = mean(SS^T * XX) + mean(TT^T * YY) - mean(2 ST^T * XY)
       = (1/Bs^2) * sum_{ij} V_i . V_j * K_ij ,  V_i = sign_i * onehot(label_i)

Distribution: row-shard X across 8 cores (512 rows each).  Each core:
 - DMA-casts its f32 shard to bf16 SBUF tiles, computes half row norms
   (ACT Square+accum), xbar-transposes its shard SBUF->SBUF into xtown,
   column-sums xtown on DVE, and writes the transposed shard to DRAM,
 - a small AllGather shares [halfsq | colsum_partial | sum(halfsq)_partial]
   so every core forms the bandwidth normalizer on device; the transposed
   shard is AllGathered in two feature-halves so Gram compute overlaps,
 - computes its Gram row panel tile-by-tile on TensorE accumulating in
   PSUM fp32 (own column block first - needs no AG data); a K=1 float32r
   matmul adds -0.5*||x_j||^2 so PSUM holds P = x_i.x_j - 0.5||x_j||^2,
 - ScalarE: E4 = exp(P * (2/sigma_4) - ||x_i||^2/sigma_4) from PSUM
   (per-partition runtime scale/bias APs); DVE squares down the chain
   E_{l-1} = E_l^2 and accumulates Ksum = sum_l E_l,
 - one tiny matmul V_blk^T @ Ksum per row tile accumulates R[c, j] in
   PSUM; per column-tile a fused DVE tensor_tensor_reduce contracts R
   with V^T,
 - partial scalar out; host sums the 8 partials and scales by 1/Bs^2.
"""

import os
from dataclasses import dataclass

import numpy as np
import ml_dtypes

import concourse.bass as bass
import concourse.bacc as bacc
import concourse.mybir as mybir
import concourse.tile as tile
from concourse.tile_rust import add_dep_helper

F32 = mybir.dt.float32
F32R = mybir.dt.float32r
BF16 = mybir.dt.bfloat16
F8 = mybir.dt.float8e4
AX = mybir.AxisListType
ALU = mybir.AluOpType
ACTF = mybir.ActivationFunctionType


@dataclass(frozen=True)
class Cfg:
    n: int = 4096          # total rows (source + target)
    d: int = 2048          # features
    cores: int = 8
    ncls: int = 8          # one-hot classes, padded 7 -> 8
    kernel_num: int = 5
    ag_split: int = 1      # big AllGather split into this many feature chunks
    use_fp8: bool = True   # fp8(e4m3) Gram with DoubleRow (2 k-tiles per MM)

    @property
    def rpc(self):  # rows per core
        return self.n // self.cores

    @property
    def ni(self):   # 128-row tiles per core
        return self.rpc // 128

    @property
    def nk(self):   # contraction (feature) tiles of 128
        return self.d // 128

    @property
    def nj(self):   # 512-wide column tiles
        return self.n // 512


CFG = Cfg()


def _build(cfg: Cfg):
    nc = bacc.Bacc(
        "TRN2",
        target_bir_lowering=False,
        debug=False,
        num_devices=cfg.cores,
    )
    NI, NK, NJ, NC = cfg.ni, cfg.nk, cfg.nj, cfg.ncls
    D, RPC, N = cfg.d, cfg.rpc, cfg.n
    NL = cfg.kernel_num
    groups = [list(range(cfg.cores))]
    AGV = RPC + D + 4
    NSPL = cfg.ag_split
    KS = NK // NSPL  # feature k-tiles per AG chunk

    xs = nc.dram_tensor("xs", [RPC, D], F32, kind="ExternalInput").ap()
    vown = nc.dram_tensor("vown", [RPC, NC], BF16, kind="ExternalInput").ap()
    vt = nc.dram_tensor("vt", [NC, N], BF16, kind="ExternalInput").ap()
    cst = nc.dram_tensor("cst", [1, 16], F32, kind="ExternalInput").ap()
    cones = nc.dram_tensor("cones", [128, 1], F32, kind="ExternalInput").ap()
    crow = nc.dram_tensor("crow", [1, 128], F32, kind="ExternalInput").ap()
    cnrow = nc.dram_tensor("cnrow", [1, 128], F32, kind="ExternalInput").ap()
    cbcol = nc.dram_tensor("cbcol", [128, 1], BF16, kind="ExternalInput").ap()
    partial = nc.dram_tensor("partial", [1, 1], F32, kind="ExternalOutput").ap()

    with tile.TileContext(nc) as tc:
        with (
            tc.tile_pool(name="dram", bufs=1, space="DRAM") as dram,
            tc.tile_pool(name="pers", bufs=1) as pers,
        ):
            XDT = F8 if cfg.use_fp8 else BF16
            shared = "Shared" if cfg.cores > 4 else "Local"
            agvec = dram.tile([AGV], F32)
            xb = dram.tile([RPC, D], BF16)
            xtd = dram.tile([D, RPC], XDT)
            xtall = [
                dram.tile([cfg.cores, D // NSPL, RPC], XDT, addr_space=shared,
                          name=f"xtall{s}")
                for s in range(NSPL)
            ]
            ag_all = dram.tile([cfg.cores * AGV], F32, addr_space=shared)

            ones_col = pers.tile([128, 1], F32)
            ones_row = pers.tile([1, 128], F32)
            negs_row = pers.tile([1, 128], F32)
            negs_rowr = pers.tile([1, 128], F32R)
            cst_sb = pers.tile([1, 16], F32)
            vown_sb = pers.tile([128, NI, NC], BF16)
            vt_sb = pers.tile([NC, N], BF16)
            halfsq = pers.tile([128, NI], F32)
            bones_col = pers.tile([128, 1], BF16)
            ag_sb = pers.tile([cfg.cores, AGV], F32)
            sc = pers.tile([128, 2 * NL], F32)
            biases = pers.tile([128, NL * NI], F32)
            loss_cols = pers.tile([NC, NJ], F32)
            lred = pers.tile([NC, 1], F32)
            out_sb = pers.tile([1, 1], F32)
            xrow = [pers.tile([128, D], BF16, name=f"xr{t}") for t in range(NI)]
            xtown = [pers.tile([128, RPC], BF16, name=f"xto{k}") for k in range(NK)]
            if cfg.use_fp8:
                # fp8 double-k tiles: [128 k_lo, 2 k_hi, cols]
                xrow8 = [pers.tile([128, D], F8, name=f"xr8{t}") for t in range(NI)]
                xtown8 = [
                    pers.tile([128, 2, RPC], F8, name=f"xto8{t}")
                    for t in range(NK // 2)
                ]
                xt8 = [
                    pers.tile([128, 2, N], F8, name=f"xt8{t}")
                    for t in range(NK // 2)
                ]
            else:
                xt = [pers.tile([128, N], BF16, name=f"xt{k}") for k in range(NK)]

            nc.sync.dma_start(ones_col[:], cones)
            nc.sync.dma_start(ones_row[:], crow)
            nc.sync.dma_start(negs_row[:], cnrow)
            nc.vector.tensor_copy(negs_rowr[:], negs_row[:])
            nc.sync.dma_start(bones_col[:], cbcol)
            nc.sync.dma_start(cst_sb[:], cst)
            nc.sync.dma_start(vown_sb[:], vown.rearrange("(i p) c -> p i c", p=128))
            nc.sync.dma_start(vt_sb[:], vt)

            # own shard: DRAM f32 -> DRAM bf16 (gpsimd cast DMA, column-chunked
            # so the transposes can start early), then to SBUF
            for q in range(4):
                nc.gpsimd.dma_start(
                    xb[:, 512 * q : 512 * (q + 1)], xs[:, 512 * q : 512 * (q + 1)]
                )
            for t in range(NI):
                nc.sync.dma_start(xrow[t][:], xb[128 * t : 128 * (t + 1), :])

            with (
                tc.tile_pool(name="pre", bufs=2) as pre,
                tc.tile_pool(name="prep", bufs=1, space="PSUM") as prep,
            ):
                # DRAM->SBUF xbar transposes into xtown
                for k in range(NK):
                    nc.sync.dma_start_transpose(
                        xtown[k][:], xb[:, 128 * k : 128 * (k + 1)]
                    )

                if cfg.use_fp8:
                    # one canonical quantization chain f32->bf16->fp8: all
                    # fp8 data (lhsT, rhs via AG, norms) derives from xb
                    for t in range(NI):
                        nc.vector.tensor_copy(xrow8[t][:], xrow[t][:])
                    for t2 in range(NK // 2):
                        for h in range(2):
                            nc.vector.tensor_copy(
                                xtown8[t2][:, h, :], xtown[2 * t2 + h][:]
                            )
                    sq_src = xrow8
                    td_src = [xtown8[k // 2][:, k % 2, :] for k in range(NK)]
                else:
                    sq_src = xrow
                    td_src = [xtown[k][:] for k in range(NK)]

                # half row norms on ACT (free-dim accumulate); column sums
                # on PE (ones matmul) into a [1, D] psum row
                psum_cs = prep.tile([1, D], F32, tag="big")
                for t in range(NI):
                    junk_sq = pre.tile([128, D], BF16, tag="junk", bufs=2)
                    nc.scalar.activation(
                        junk_sq[:],
                        sq_src[t][:],
                        ACTF.Square,
                        scale=float(np.sqrt(0.5)),
                        accum_out=halfsq[:, t : t + 1],
                    )
                    for ch in range(D // 512):
                        nc.tensor.matmul(
                            psum_cs[:, 512 * ch : 512 * (ch + 1)],
                            lhsT=bones_col[:],
                            rhs=xrow[t][:, 512 * ch : 512 * (ch + 1)],
                            start=(t == 0),
                            stop=(t == NI - 1),
                        )

                # partial sum of halfsq (partition reduce via ones matmul)
                psum_hs = prep.tile([1, NI], F32, tag="small")
                nc.tensor.matmul(
                    psum_hs[:], lhsT=ones_col[:], rhs=halfsq[:], start=True, stop=True
                )

                # assemble the small-AG vector in DRAM
                nc.gpsimd.dma_start(
                    agvec[0:RPC].rearrange("(t p) -> p t", p=128), halfsq[:]
                )
                sbvec = pre.tile([1, D + 4], F32, tag="sbvec", bufs=1)
                nc.vector.tensor_copy(sbvec[:, 0:D], psum_cs[:])
                nc.vector.tensor_copy(sbvec[:, D : D + NI], psum_hs[:])
                nc.gpsimd.dma_start(
                    agvec[RPC : RPC + D + 4].rearrange("(o c) -> o c", o=1),
                    sbvec[:],
                )

                # write the transposed shard back to DRAM (big-AG input)
                for k in range(NK):
                    nc.sync.dma_start(xtd[128 * k : 128 * (k + 1), :], td_src[k])
                # big AG first (it gates the Gram and absorbs the cross-core
                # start skew); the small bandwidth AG follows -- its result
                # is only needed by the first exp, well after the first
                # k-loop finishes
                ag_big = []
                for s in range(NSPL):
                    cc = nc.gpsimd.collective_compute(
                        "AllGather",
                        ALU.bypass,
                        replica_groups=groups,
                        ins=[xtd[s * (D // NSPL) : (s + 1) * (D // NSPL), :].opt()],
                        outs=[xtall[s][:, :, :].opt()],
                    )
                    ag_big.append(cc)
                for s in range(1, NSPL):
                    add_dep_helper(
                        ag_big[s].ins, ag_big[s - 1].ins, sync=False,
                        reason="AG chunks in order",
                    )
                ag_small = nc.gpsimd.collective_compute(
                    "AllGather",
                    ALU.bypass,
                    replica_groups=groups,
                    ins=[agvec[:].opt()],
                    outs=[ag_all[:].opt()],
                )
                add_dep_helper(
                    ag_small.ins, ag_big[-1].ins, sync=False,
                    reason="big AG first, small after",
                )

                nc.scalar.dma_start(
                    ag_sb[:], ag_all[:].rearrange("(r c) -> r c", c=AGV)
                )

                # bandwidth: s1 = sum halfsq partials, s2 = ||sum_i x_i||^2
                psum_s = prep.tile([1, NI], F32, tag="small")
                nc.tensor.matmul(
                    psum_s[:],
                    lhsT=ones_col[0 : cfg.cores, :],
                    rhs=ag_sb[:, RPC + D : RPC + D + NI],
                    start=True,
                    stop=True,
                )
                s1 = pre.tile([1, 1], F32, tag="tiny", bufs=8)
                nc.vector.tensor_reduce(s1[:], psum_s[:], axis=AX.X, op=ALU.add)
                s2c = pre.tile([1, 4], F32, tag="s2c", bufs=1)
                for ch in range(4):
                    psum_cg = prep.tile([1, 512], F32, tag="cg", bufs=2)
                    nc.tensor.matmul(
                        psum_cg[:],
                        lhsT=ones_col[0 : cfg.cores, :],
                        rhs=ag_sb[:, RPC + 512 * ch : RPC + 512 * (ch + 1)],
                        start=True,
                        stop=True,
                    )
                    junk_cg = pre.tile([1, 512], BF16, tag="junkcg", bufs=2)
                    nc.scalar.activation(
                        junk_cg[:], psum_cg[:], ACTF.Square,
                        accum_out=s2c[:, ch : ch + 1],
                    )
                s2 = pre.tile([1, 1], F32, tag="tiny", bufs=8)
                nc.vector.tensor_reduce(s2[:], s2c[:], axis=AX.X, op=ALU.add)
                t1 = pre.tile([1, 1], F32, tag="tiny", bufs=8)
                t2 = pre.tile([1, 1], F32, tag="tiny", bufs=8)
                bw0 = pre.tile([1, 1], F32, tag="tiny", bufs=8)
                inv0 = pre.tile([1, 1], F32, tag="tiny", bufs=8)
                nc.vector.tensor_scalar_mul(t1[:], s1[:], 1.0 / (N - 1))
                nc.vector.tensor_scalar_mul(t2[:], s2[:], -1.0 / (2.0 * N * (N - 1)))
                nc.vector.tensor_tensor(bw0[:], t1[:], t2[:], op=ALU.add)
                nc.vector.reciprocal(inv0[:], bw0[:])

                # per-partition scale / bias APs for the exp
                sc10 = pre.tile([1, 2 * NL], F32, tag="sc10", bufs=1)
                nc.vector.tensor_scalar_mul(sc10[:], cst_sb[:, 0 : 2 * NL], inv0[:])
                psum_b = prep.tile([128, 2 * NL], F32, tag="small")
                nc.tensor.matmul(
                    psum_b[:], lhsT=ones_row[:], rhs=sc10[:], start=True, stop=True
                )
                nc.vector.tensor_copy(sc[:], psum_b[:])
                for l in range(NL):
                    nc.vector.tensor_scalar_mul(
                        biases[:, NI * l : NI * (l + 1)],
                        halfsq[:],
                        sc[:, NL + l : NL + l + 1],
                    )

            # gather the full X^T into SBUF (k-major)
            for s in range(NSPL):
                for kk in range(KS):
                    k = s * KS + kk
                    if cfg.use_fp8:
                        dst = xt8[k // 2][:, k % 2, :]
                    else:
                        dst = xt[k][:]
                    nc.sync.dma_start(
                        dst.rearrange("p (r c) -> p r c", r=cfg.cores),
                        xtall[s][:, 128 * kk : 128 * (kk + 1), :].rearrange(
                            "r p c -> p r c"
                        ),
                    )

            with (
                tc.tile_pool(name="work", bufs=2) as work,
                tc.tile_pool(name="mpsum", bufs=1, space="PSUM") as mpsum,
            ):
                LAST = NL - 1

                def do_jt(jt):
                    hsj32 = work.tile([1, 512], F32, tag="hsj32", bufs=2)
                    hsj = work.tile([1, 512], F32R, tag="hsj", bufs=2)
                    src = ag_all[AGV * jt : AGV * jt + 512]
                    nc.scalar.dma_start(
                        hsj32[:], src.rearrange("(o c) -> o c", o=1)
                    )
                    nc.vector.tensor_copy(hsj[:], hsj32[:])
                    psum_R = mpsum.tile([NC, 512], F32, tag="R", bufs=2)
                    gs = [
                        mpsum.tile([128, 512], F32, tag="g", bufs=6,
                                   name=f"g_{jt}_{i}")
                        for i in range(NI)
                    ]
                    if cfg.use_fp8:
                        for t2 in range(NK // 2):
                            rhs = xt8[t2][:, :, 512 * jt : 512 * (jt + 1)]
                            for i in range(NI):
                                nc.tensor.matmul(
                                    gs[i],
                                    lhsT=xtown8[t2][:, :, 128 * i : 128 * (i + 1)],
                                    rhs=rhs,
                                    start=(t2 == 0),
                                    stop=False,
                                    perf_mode=mybir.MatmulPerfMode.DoubleRow,
                                )
                    else:
                        for k in range(NK):
                            rhs = xt[k][:, 512 * jt : 512 * (jt + 1)]
                            for i in range(NI):
                                nc.tensor.matmul(
                                    gs[i],
                                    lhsT=xtown[k][:, 128 * i : 128 * (i + 1)],
                                    rhs=rhs,
                                    start=(k == 0),
                                    stop=False,
                                )
                    for i in range(NI):
                        nc.tensor.matmul(
                            gs[i],
                            lhsT=negs_rowr[:],
                            rhs=hsj[:],
                            start=False,
                            stop=True,
                        )
                    for i in range(NI):
                        # E4 from PSUM; square down the chain E_{l-1} = E_l^2
                        # with the work spread across ACT / DVE / GpSimd so no
                        # single engine bottlenecks:
                        #   ACT: exp, E3, E2    DVE: E1, E0, S3, S4
                        #   GpSimd: S1, S2
                        E4 = work.tile([128, 512], BF16, tag="E", bufs=3)
                        nc.scalar.activation(
                            E4[:],
                            gs[i][:],
                            ACTF.Exp,
                            bias=biases[:, NI * LAST + i : NI * LAST + i + 1],
                            scale=sc[:, LAST : LAST + 1],
                        )
                        E3 = work.tile([128, 512], BF16, tag="T", bufs=4)
                        nc.scalar.activation(E3[:], E4[:], ACTF.Square)
                        E2 = work.tile([128, 512], BF16, tag="T", bufs=4)
                        nc.scalar.activation(E2[:], E3[:], ACTF.Square)
                        E1 = work.tile([128, 512], BF16, tag="T", bufs=4)
                        nc.vector.tensor_tensor(E1[:], E2[:], E2[:], op=ALU.mult)
                        E0 = work.tile([128, 512], BF16, tag="T", bufs=4)
                        nc.vector.tensor_tensor(E0[:], E1[:], E1[:], op=ALU.mult)
                        S1 = work.tile([128, 512], BF16, tag="S", bufs=4)
                        nc.gpsimd.tensor_tensor(S1[:], E4[:], E3[:], op=ALU.add)
                        S2 = work.tile([128, 512], BF16, tag="S", bufs=4)
                        nc.gpsimd.tensor_tensor(S2[:], S1[:], E2[:], op=ALU.add)
                        S3 = work.tile([128, 512], BF16, tag="S", bufs=4)
                        nc.vector.tensor_tensor(S3[:], S2[:], E1[:], op=ALU.add)
                        S4 = work.tile([128, 512], BF16, tag="S", bufs=4)
                        nc.vector.tensor_tensor(S4[:], S3[:], E0[:], op=ALU.add)
                        nc.tensor.matmul(
                            psum_R,
                            lhsT=vown_sb[:, i, :],
                            rhs=S4[:],
                            start=(i == 0),
                            stop=(i == NI - 1),
                        )

                    scr = work.tile([NC, 512], F32, tag="scr", bufs=2)
                    nc.vector.tensor_tensor(
                        scr[:],
                        psum_R[:],
                        vt_sb[:, 512 * jt : 512 * (jt + 1)],
                        op=ALU.mult,
                    )
                    nc.vector.tensor_reduce(
                        loss_cols[:, jt : jt + 1], scr[:], axis=AX.X, op=ALU.add
                    )

                for jt in range(NJ):
                    do_jt(jt)

                nc.vector.tensor_reduce(
                    lred[:], loss_cols[:, 0:NJ], axis=AX.X, op=ALU.add
                )
                psum_f = mpsum.tile([1, 1], F32, tag="R", bufs=2)
                nc.tensor.matmul(
                    psum_f[:],
                    lhsT=lred[:],
                    rhs=ones_col[0:NC, :],
                    start=True,
                    stop=True,
                )
                nc.vector.tensor_copy(out_sb[:], psum_f[:])
                nc.sync.dma_start(partial, out_sb[:])

    nc.compile()
    return nc


def host_prep(cfg: Cfg, source, target, s_label, t_label):
    """Slice/encode inputs into per-core in_maps (no arithmetic on X)."""
    X = np.concatenate(
        [np.asarray(source, np.float32), np.asarray(target, np.float32)], 0
    )
    bs = np.asarray(source).shape[0]
    lab = np.concatenate(
        [np.asarray(s_label).astype(np.int64), np.asarray(t_label).astype(np.int64)]
    )
    sign = np.ones(cfg.n, np.float32)
    sign[bs:] = -1.0
    V = np.zeros((cfg.n, cfg.ncls), np.float32)
    V[np.arange(cfg.n), lab] = sign
    Vb = V.astype(ml_dtypes.bfloat16)
    VtB = np.ascontiguousarray(V.T).astype(ml_dtypes.bfloat16)

    NL = cfg.kernel_num
    cst = np.zeros((1, 16), np.float32)
    for l in range(NL):
        cst[0, l] = 2.0 * 2.0 ** (-l)        # scale_l * bw:  2 / (2^l)
        cst[0, NL + l] = -2.0 * 2.0 ** (-l)  # bias mult_l * bw / halfsq
    cones = np.ones((128, 1), np.float32)
    crow = np.ones((1, 128), np.float32)
    cnrow = -np.ones((1, 128), np.float32)
    cbcol = np.ones((128, 1), ml_dtypes.bfloat16)

    in_maps = []
    for c in range(cfg.cores):
        r0, r1 = c * cfg.rpc, (c + 1) * cfg.rpc
        in_maps.append(
            {
                "xs": np.ascontiguousarray(X[r0:r1]),
                "vown": np.ascontiguousarray(Vb[r0:r1]),
                "vt": VtB,
                "cst": cst,
                "cones": cones,
                "crow": crow,
                "cnrow": cnrow,
                "cbcol": cbcol,
            }
        )
    return in_maps


_NC_CACHE = {}


def _get_nc(cfg: Cfg):
    key = cfg
    if key not in _NC_CACHE:
        _NC_CACHE[key] = _build(cfg)
    return _NC_CACHE[key]


def run(inputs: dict, cfg: Cfg = CFG, trace: bool = False):
    from concourse.bass_utils import run_bass_kernel_spmd

    nc = _get_nc(cfg)
    in_maps = host_prep(
        cfg,
        inputs["source"],
        inputs["target"],
        inputs["s_label"],
        inputs["t_label"],
    )
    res = run_bass_kernel_spmd(
        nc, in_maps, core_ids=list(range(cfg.cores)), trace=trace
    )
    bs = np.asarray(inputs["source"]).shape[0]
    total = sum(float(r["partial"][0, 0]) for r in res.results)
    loss = np.float32(total / float(bs) ** 2)
    return np.asarray(loss, dtype=np.float32), res


def kernel(**inputs) -> np.ndarray:
    out, _ = run(inputs)
    return out
